# revision 13
# baseline (speedup 1.0000x reference)
"""Trainium2 Bass kernel for the EnrichClassifier pathway MLP.

Network (eval mode, BN folded into weights):
  h1 = relu(x @ (w1*m1).T * s1 + b1')   [8192,5000] -> [8192,4000]
  h2 = relu(h1 @ (w2*m2).T * s2 + b2')                 -> [8192,2000]
  h3 = relu(h2 @ (w3*m3).T * s3 + b3')                 -> [8192,1000]
  sc = relu(h3 @ (w4*m4).T + b4)                       -> [8192,200]
  out = sc @ wc.T + bc                                 -> [8192,50]

Structure: m1 gives each of 200 pathways a private set of 100 genes;
20 L1 units per pathway share that set. m2/m3/m4 are block-diagonal
(20->10->5->1 per pathway). Effective work ~7.5 GFLOP vs 495 dense.

L1 strategy: x is pre-gathered on the HOST into fp8. Pathways are
paired into 17 supergroups of 12 (two 120-unit h1 tiles each); the
supergroup's 1200 genes are concatenated into 5 chunks of 256 slots.
Each chunk is one fp8 DoubleRow matmul (2x128 contraction slots per
pass, half streaming cost); a chunk straddling the two h1 tiles is
issued once per tile. No on-device gather.

Sharding: pure data parallel over batch across the 8 cores (1024 rows
per core); packed weights replicated.
"""

import contextlib
import os

import numpy as np

import concourse.bass as bass
import concourse.bacc as bacc
import concourse.tile as tile
import concourse.mybir as mybir
from concourse.bass_utils import run_bass_kernel_spmd

# ---------------- hardcoded geometry ----------------
B, G, NPATH = 8192, 5000, 200
NCORES = 8
BC = B // NCORES            # 1024 rows per core
NT = 2                      # batch tiles per core
NB = BC // NT               # 512 = PSUM bank free size (fp32)
U1, U2, U3 = 20, 10, 5      # per-pathway units per layer
NL = 50                     # labels
KGENES = 100                # genes per pathway
GP = 6                      # pathways per h1 tile (120 units)
NG = 34                     # h1 tiles/groups (33 full + 1 of 2)
NSG = 17                    # supergroups: 12 pathways = 2 h1 tiles
NPR = 9                     # h3 tiles (2 supergroups each)
MAXBLK = 10                 # 128-slot blocks per supergroup (sg16: 8)
W1SCALE = 64.0              # fp8 upscale of w1; undone via w2 downscale
F32 = mybir.dt.float32
F32R = mybir.dt.float32r
FP8 = mybir.dt.float8e4
NP_FP8 = mybir.dt.np(FP8)
RELU = mybir.ActivationFunctionType.Relu
IDENT = mybir.ActivationFunctionType.Identity
DROW = mybir.MatmulPerfMode.DoubleRow

_COMPILED = None  # cached compiled program across calls


def _sg_paths(sg):
    return range(12 * sg, min(12 * sg + 12, NPATH))


def _sg_nslots(sg):
    return len(_sg_paths(sg)) * KGENES     # 1200 (sg16: 800)


def _sg_nchunks(sg):
    return (_sg_nslots(sg) + 255) // 256   # 5 (sg16: 4)


def _sg_mms(sg):
    """Issue-ordered (chunk k, side) DoubleRow matmuls for supergroup sg.

    Side 0 = first 6 pathways (slots [0,600)), side 1 = rest."""
    n = _sg_nslots(sg)
    out = []
    for side, (lo, hi) in enumerate([(0, 600), (600, n)]):
        for k in range(_sg_nchunks(sg)):
            if 256 * k < hi and 256 * (k + 1) > lo:
                out.append((k, side))
    return out


def _mm_plan():
    """Global enumeration of L1 matmuls -> w1s column base."""
    col = {}
    mi = 0
    for sg in range(NSG):
        for k, side in _sg_mms(sg):
            col[(sg, k, side)] = 256 * mi
            mi += 1
    return col, mi


MMCOL, NMM1 = _mm_plan()   # 101 matmuls


def _pack_static(inputs):
    """Pack weights/biases (shared across cores). Pure layout/folding."""
    f = lambda k: np.asarray(inputs[k], np.float32)
    w1, b1, m1 = f("w1"), f("b1"), f("m1")
    b2, b3, b4 = f("b2"), f("b3"), f("b4")
    wc, bc = f("wc"), f("bc")

    def fold(gamma, beta, rm, rv):
        s = gamma / np.sqrt(rv + 1e-5)
        return s, beta - rm * s

    s1, t1 = fold(f("gamma1"), f("beta1"), f("rm1"), f("rv1"))
    s2, t2 = fold(f("gamma2"), f("beta2"), f("rm2"), f("rv2"))
    s3, t3 = fold(f("gamma3"), f("beta3"), f("rm3"), f("rv3"))
    w1m = w1 * m1 * s1[:, None]
    b1f = b1 * s1 + t1
    w2m = f("w2") * f("m2") * s2[:, None]
    b2f = b2 * s2 + t2
    w3m = f("w3") * f("m3") * s3[:, None]
    b3f = b3 * s3 + t3
    w4m = f("w4") * f("m4")

    # pathway p -> its gene rows (from m1's structure)
    genes = [np.nonzero(m1[U1 * p] != 0)[0] for p in range(NPATH)]
    for g in genes:
        assert len(g) == KGENES

    # global gathered slot table: supergroup sg at slots [1280sg, +1280)
    slot_gene = np.zeros(NSG * 128 * MAXBLK, np.int64)
    sg_lists = []
    for sg in range(NSG):
        lg = np.concatenate([genes[p] for p in _sg_paths(sg)])
        sg_lists.append(lg)
        slot_gene[1280 * sg : 1280 * sg + len(lg)] = lg

    # L1 DoubleRow stationary fp8 [128, NMM1*256]: matmul m=(sg,k,side)
    # at cols [256m, +256) viewed as [128 part, 2 ko, 128 unit]:
    # [r, j, 20*(pi%6)+u] = w1m_scaled[unit u of pathway pi, gene(slot)]
    # where slot = 256k+128j+r belongs to pathway pi of side's h1 tile.
    w1s = np.zeros((128, NMM1 * 256), np.float32)
    b1v = np.zeros((128, NG), np.float32)
    for sg in range(NSG):
        paths = list(_sg_paths(sg))
        lg = sg_lists[sg]
        for k, side in _sg_mms(sg):
            base = MMCOL[(sg, k, side)]
            for j in range(2):
                for r in range(128):
                    s = 256 * k + 128 * j + r
                    if s >= len(lg):
                        continue
                    pi = s // KGENES
                    if pi // GP != side:
                        continue
                    p = paths[pi]
                    u0 = U1 * (pi % GP)
                    w1s[r, base + 128 * j + u0 : base + 128 * j + u0 + U1] = (
                        w1m[U1 * p : U1 * p + U1, lg[s]] * W1SCALE)
    for g in range(NG):
        for pi, p in enumerate(range(GP * g, min(GP * g + GP, NPATH))):
            b1v[U1 * pi : U1 * pi + U1, g] = b1f[U1 * p : U1 * p + U1] * W1SCALE

    # L2 stationary f32r [128, NG*128]: h1 tile g rows (20*pi+u) ->
    # h2 supergroup tile sg=g//2 rows (10*qi+v); w2 divided by W1SCALE
    # to undo the fp8 upscale of h1.
    w2s = np.zeros((128, NG * 128), np.float32)
    b2v = np.zeros((128, NSG), np.float32)
    for g in range(NG):
        sg = g // 2
        for pi, p in enumerate(range(GP * g, min(GP * g + GP, NPATH))):
            qi = p - 12 * sg
            blk = w2m[U2 * p : U2 * p + U2, U1 * p : U1 * p + U1] / W1SCALE
            w2s[U1 * pi : U1 * pi + U1,
                128 * g + U2 * qi : 128 * g + U2 * qi + U2] = blk.T
    for sg in range(NSG):
        for qi, p in enumerate(_sg_paths(sg)):
            b2v[U2 * qi : U2 * qi + U2, sg] = b2f[U2 * p : U2 * p + U2]

    # L3 stationary f32r [128, NSG*128]: supergroup sg rows (10*qi+v) ->
    # h3 tile pr=sg//2 rows (5*ri+w).
    w3s = np.zeros((128, NSG * 128), np.float32)
    b3v = np.zeros((128, NPR), np.float32)
    for sg in range(NSG):
        pr = sg // 2
        for p in _sg_paths(sg):
            qi = p - 12 * sg
            ri = p - 24 * pr
            blk = w3m[U3 * p : U3 * p + U3, U2 * p : U2 * p + U2]
            w3s[U2 * qi : U2 * qi + U2,
                128 * sg + U3 * ri : 128 * sg + U3 * ri + U3] = blk.T
    for pr in range(NPR):
        for p in range(24 * pr, min(24 * pr + 24, NPATH)):
            ri = p - 24 * pr
            b3v[U3 * ri : U3 * ri + U3, pr] = b3f[U3 * p : U3 * p + U3]

    # L4 stationary f32r [128, NPR*128]: h3 tile pr rows (5*ri+w) ->
    # scores tile T=0 (pathways 0-119) or T=1 (120-199), row p-120T.
    w4s = np.zeros((128, NPR * 128), np.float32)
    b4v = np.zeros((128, 2), np.float32)
    for pr in range(NPR):
        T = 0 if pr < 5 else 1
        for p in range(24 * pr, min(24 * pr + 24, NPATH)):
            ri = p - 24 * pr
            w4s[U3 * ri : U3 * ri + U3, 128 * pr + p - 120 * T] = (
                w4m[p, U3 * p : U3 * p + U3])
    b4v[:120, 0] = b4[:120]
    b4v[:80, 1] = b4[120:]

    # classifier stationary [128, 2*64]: rows = scores-tile rows, cols labels
    wcs = np.zeros((128, 2 * 64), np.float32)
    wcs[:120, :NL] = wc[:, :120].T
    wcs[:80, 64 : 64 + NL] = wc[:, 120:].T
    bcv = np.zeros((128, 1), np.float32)
    bcv[:NL, 0] = bc

    ident = np.eye(64, dtype=np.float32)

    shared = {
        "w1s": np.ascontiguousarray(w1s, dtype=NP_FP8),
        "w2s": w2s, "w3s": w3s, "w4s": w4s, "wcs": wcs,
        "b1v": b1v, "b2v": b2v, "b3v": b3v, "b4v": b4v, "bcv": bcv,
        "ident": ident,
    }
    return shared, slot_gene


def _pack(inputs):
    """Host-side packing: folded weights + per-core pre-gathered fp8 x."""
    shared, slot_gene = _pack_static(inputs)

    x8 = np.asarray(np.asarray(inputs["x"], np.float32), NP_FP8)
    xt8 = np.ascontiguousarray(x8.T)               # [G, B] fp8
    xg_all = xt8[slot_gene]                        # [NSG*1280, B]
    # -> per core [NT, NSG, 128, MAXBLK*NB]
    xg6 = xg_all.reshape(NSG, MAXBLK, 128, NCORES, NT, NB)
    in_maps = []
    for c in range(NCORES):
        m = dict(shared)
        m["xg"] = np.ascontiguousarray(
            xg6[:, :, :, c].transpose(3, 0, 2, 1, 4)).reshape(
                NT, NSG, 128, MAXBLK * NB)
        in_maps.append(m)
    return in_maps


def _build(repeat=None):
    """Build + compile the per-core Bass program (shared across cores).

    repeat: if set, wrap the whole compute body in an on-device For_i loop
    (used only for timing measurements; outputs are identical)."""
    nc = bacc.Bacc("TRN2", target_bir_lowering=False, debug=False,
                   enable_asserts=False)

    dram_in = {}
    for name, shape, dt_ in [
        ("xg", [NT, NSG, 128, MAXBLK * NB], FP8),
        ("w1s", [128, NMM1 * 256], FP8),
        ("w2s", [128, NG * 128], F32R), ("w3s", [128, NSG * 128], F32R),
        ("w4s", [128, NPR * 128], F32R), ("wcs", [128, 2 * 64], F32R),
        ("b1v", [128, NG], F32), ("b2v", [128, NSG], F32),
        ("b3v", [128, NPR], F32), ("b4v", [128, 2], F32),
        ("bcv", [128, 1], F32), ("ident", [64, 64], F32),
    ]:
        dram_in[name] = nc.dram_tensor(name, shape, dt_, kind="ExternalInput").ap()
    out_d = nc.dram_tensor("out", [BC, NL], F32, kind="ExternalOutput").ap()

    with tile.TileContext(nc) as tc:
        const = tc.alloc_tile_pool(name="const", bufs=1, space="SBUF")
        cs = {}
        for name, ap in dram_in.items():
            if name == "xg":
                continue
            t = const.tile(ap.shape, ap.dtype, name=f"c_{name}")
            nc.sync.dma_start(t[:], ap[:])
            cs[name] = t
        if os.environ.get("DIAG_NO_DMA"):
            t = const.tile([128, MAXBLK * NB], FP8, name="c_diag_gt")
            nc.sync.dma_start(t[:], dram_in["xg"][0, 0])
            cs["diag_gt"] = t

        gpool = tc.alloc_tile_pool(name="gath", bufs=3, space="SBUF")
        h1p = tc.alloc_tile_pool(name="h1", bufs=3, space="SBUF")
        h2p = tc.alloc_tile_pool(name="h2", bufs=3, space="SBUF")
        h3p = tc.alloc_tile_pool(name="h3", bufs=2, space="SBUF")
        scp = tc.alloc_tile_pool(name="sc", bufs=3, space="SBUF")
        otp = tc.alloc_tile_pool(name="ot", bufs=2, space="SBUF")
        osb = tc.alloc_tile_pool(name="osb", bufs=2, space="SBUF")
        ps1 = tc.alloc_tile_pool(name="ps1", bufs=3, space="PSUM")
        ps2 = tc.alloc_tile_pool(name="ps2", bufs=1, space="PSUM")
        ps3 = tc.alloc_tile_pool(name="ps3", bufs=1, space="PSUM")
        ps4 = tc.alloc_tile_pool(name="ps4", bufs=1, space="PSUM")
        psc = tc.alloc_tile_pool(name="psc", bufs=1, space="PSUM")
        pst = tc.alloc_tile_pool(name="pst", bufs=1, space="PSUM")

        def bias_relu(dst, src, bias, on_vector):
            if os.environ.get("DIAG_TINY_ACT"):
                dst, src = dst[:, :32], src[:, :32]
            if on_vector:
                nc.vector.tensor_scalar(dst, src, bias, 0.0,
                                        mybir.AluOpType.add,
                                        mybir.AluOpType.max)
            else:
                nc.scalar.activation(dst, src, RELU, bias=bias)

        loop_kw = {}
        if os.environ.get("LOOP_STAGGER"):
            loop_kw["staggered_reset"] = True
        if os.environ.get("LOOP_HINT"):
            from concourse.engine_type import EngineType
            loop_kw["hint_engines"] = (EngineType.PE, EngineType.DVE,
                                       EngineType.Activation, EngineType.SP)
        loop = (tc.For_i(0, repeat, 1, **loop_kw) if repeat
                else contextlib.nullcontext())
        with loop:
            for nt in range(NT):
                sc_tiles = []
                for sg in range(NSG):
                    nblk = 2 * _sg_nchunks(sg)
                    if os.environ.get("DIAG_NO_DMA"):
                        gt = cs["diag_gt"]
                    else:
                        gt = gpool.tile([128, MAXBLK * NB], FP8, name="gt", tag="gt")
                        nc.sync.dma_start(gt[:, : nblk * NB],
                                          dram_in["xg"][nt, sg, :, : nblk * NB])
                    mms = _sg_mms(sg)
                    p2 = ps2.tile([128, NB], F32, name="p2", tag="p2")
                    for side in range(2):
                        g = 2 * sg + side
                        ks = [k for k, s in mms if s == side]
                        if os.environ.get("DIAG_L1_HALF"):
                            ks = ks[: max(1, len(ks) // 2)]
                        # ---- L1: fp8 DoubleRow over 256-slot chunks ----
                        p1 = ps1.tile([128, NB], F32, name="p1", tag="p1")
                        for i, k in enumerate(ks):
                            cb = 0 if os.environ.get("DIAG_FIXED_W") else (
                                MMCOL[(sg, k, side)])
                            lhsT = cs["w1s"][:, cb : cb + 256].rearrange(
                                "p (two u) -> p two u", two=2)
                            rhs = gt[:, 2 * k * NB : (2 * k + 2) * NB].rearrange(
                                "p (two n) -> p two n", two=2)
                            nc.tensor.matmul(
                                p1[:], lhsT, rhs, perf_mode=DROW,
                                start=(i == 0), stop=(i == len(ks) - 1),
                            )
                        h1 = h1p.tile([128, NB], F32R, name="h1t", tag="h1t")
                        bias_relu(h1[:], p1[:], cs["b1v"][:, g : g + 1], g % 2)
                        # ---- L2: accumulate the supergroup's 2 tiles ----
                        nc.tensor.matmul(
                            p2[:], cs["w2s"][:, 128 * g : 128 * (g + 1)],
                            h1[:], start=(side == 0), stop=(side == 1),
                        )
                    h2 = h2p.tile([128, NB], F32R, name="h2t", tag="h2t")
                    bias_relu(h2[:], p2[:], cs["b2v"][:, sg : sg + 1], sg % 2)
                    # ---- L3: accumulate pairs of supergroups ----
                    if sg % 2 == 0:
                        p3 = ps3.tile([128, NB], F32, name="p3", tag="p3")
                    last3 = (sg % 2 == 1) or (sg == NSG - 1)
                    nc.tensor.matmul(
                        p3[:], cs["w3s"][:, 128 * sg : 128 * (sg + 1)],
                        h2[:], start=(sg % 2 == 0), stop=last3,
                    )
                    if last3:
                        pr = sg // 2
                        h3 = h3p.tile([128, NB], F32R, name="h3t", tag="h3t")
                        bias_relu(h3[:], p3[:], cs["b3v"][:, pr : pr + 1], pr % 2)
                        # ---- L4: scores tile A (pr 0-4) / B (pr 5-8) ----
                        T = 0 if pr < 5 else 1
                        first4 = pr in (0, 5)
                        if first4:
                            p4 = ps4.tile([128, NB], F32, name="p4", tag="p4")
                        nc.tensor.matmul(
                            p4[:], cs["w4s"][:, 128 * pr : 128 * (pr + 1)],
                            h3[:], start=first4, stop=(pr in (4, NPR - 1)),
                        )
                        if pr in (4, NPR - 1):
                            sc = scp.tile([128, NB], F32R, name="sct", tag="sct")
                            bias_relu(sc[:], p4[:], cs["b4v"][:, T : T + 1], T)
                            sc_tiles.append((T, sc))
                # ---- classifier ----
                pc = psc.tile([128, NB], F32, name="pc", tag="pc")
                for k, (T, sc) in enumerate(sc_tiles):
                    nc.tensor.matmul(
                        pc[:64, :], cs["wcs"][:, 64 * T : 64 * (T + 1)],
                        sc[:], start=(k == 0), stop=(k == len(sc_tiles) - 1),
                    )
                ot = otp.tile([64, NB], F32, name="ott", tag="ott")
                nc.scalar.activation(ot[:], pc[:64, :], IDENT, bias=cs["bcv"][:64, 0:1])
                # ---- transpose [64, 512] -> 4 x [128, 64] and store ----
                ob = osb.tile([128, 4 * 64], F32, name="obt", tag="obt")
                for c in range(4):
                    pt = pst.tile([128, 64], F32, name="ptt", tag="ptt")
                    nc.tensor.transpose(pt[:], ot[:, 128 * c : 128 * (c + 1)],
                                        cs["ident"][:])
                    nc.vector.tensor_copy(ob[:, 64 * c : 64 * (c + 1)], pt[:])
                dst = out_d[NB * nt : NB * (nt + 1), :].rearrange(
                    "(c p) l -> p c l", p=128)
                nc.sync.dma_start(dst, ob.rearrange("p (c l) -> p c l", c=4)[:, :, :NL])

        for pl in (pst, psc, ps4, ps3, ps2, ps1, osb, otp, scp,
                   h3p, h2p, h1p, gpool, const):
            pl.release()

    nc.compile()
    return nc


def get_compiled():
    global _COMPILED
    if _COMPILED is None:
        _COMPILED = _build()
    return _COMPILED


def kernel(**inputs):
    nc = get_compiled()
    in_maps = _pack(inputs)
    res = run_bass_kernel_spmd(nc, in_maps, core_ids=list(range(NCORES)))
    return np.concatenate([res.results[c]["out"] for c in range(NCORES)], axis=0)


if __name__ == "__main__":
    print("built", get_compiled())


# revision 18
# speedup vs baseline: 1.4241x; 1.4241x over previous
"""Trainium2 Bass kernel for the EnrichClassifier pathway MLP.

Network (eval mode, BN folded into weights):
  h1 = relu(x @ (w1*m1).T * s1 + b1')   [8192,5000] -> [8192,4000]
  h2 = relu(h1 @ (w2*m2).T * s2 + b2')                 -> [8192,2000]
  h3 = relu(h2 @ (w3*m3).T * s3 + b3')                 -> [8192,1000]
  sc = relu(h3 @ (w4*m4).T + b4)                       -> [8192,200]
  out = sc @ wc.T + bc                                 -> [8192,50]

Structure: m1 gives each of 200 pathways a private set of 100 genes;
20 L1 units per pathway share that set. m2/m3/m4 are block-diagonal
(20->10->5->1 per pathway). Effective work ~7.5 GFLOP vs 495 dense.

L1: x is pre-gathered on the HOST into fp8. Pathways are paired into
17 supergroups of 12 (two 120-unit h1 tiles); the supergroup's 1200
genes are concatenated into 5 chunks of 256 slots, each one fp8
DoubleRow matmul (2x128 contraction per pass). h1 is written as fp8
(scaled x64) so L2 is a single DoubleRow matmul per supergroup; h2/h3/
scores flow in bf16 with scales folded into the next layer's weights.
The two 512-row batch halves are interleaved through the whole pipeline
to keep every engine fed. Output leaves the device label-major
[NT, 64, NB]; the host transposes.

Sharding: pure data parallel over batch across the 8 cores (1024 rows
per core); packed weights replicated.
"""

import contextlib
import os

import numpy as np

import concourse.bass as bass
import concourse.bacc as bacc
import concourse.tile as tile
import concourse.mybir as mybir
from concourse.bass_utils import run_bass_kernel_spmd

# ---------------- hardcoded geometry ----------------
B, G, NPATH = 8192, 5000, 200
NCORES = 8
BC = B // NCORES            # 1024 rows per core
NT = 2                      # batch tiles per core
NB = BC // NT               # 512 = PSUM bank free size (fp32)
U1, U2, U3 = 20, 10, 5      # per-pathway units per layer
NL = 50                     # labels
KGENES = 100                # genes per pathway
GP = 6                      # pathways per h1 tile (120 units)
NG = 34                     # h1 tiles/groups (33 full + 1 of 2)
NSG = 17                    # supergroups: 12 pathways = 2 h1 tiles
NPR = 9                     # h3 tiles (2 supergroups each)
MAXBLK = 10                 # 128-slot blocks per supergroup (sg16: 8)
W1SCALE = 64.0              # fp8 upscale of w1 (h1 stored = 64*h1_true)
W2SCALE = 64.0              # fp8 upscale of w2 (h2 stored = 4096*h2_true)
H2SCALE = W1SCALE * W2SCALE
F32 = mybir.dt.float32
F32R = mybir.dt.float32r
BF16 = mybir.dt.bfloat16
FP8 = mybir.dt.float8e4
NP_FP8 = mybir.dt.np(FP8)
RELU = mybir.ActivationFunctionType.Relu
IDENT = mybir.ActivationFunctionType.Identity
DROW = mybir.MatmulPerfMode.DoubleRow

_COMPILED = None  # cached compiled program across calls


def _sg_paths(sg):
    return range(12 * sg, min(12 * sg + 12, NPATH))


def _sg_nslots(sg):
    return len(_sg_paths(sg)) * KGENES     # 1200 (sg16: 800)


def _sg_nchunks(sg):
    return (_sg_nslots(sg) + 255) // 256   # 5 (sg16: 4)


def _sg_mms(sg):
    """Issue-ordered (chunk k, side) DoubleRow matmuls for supergroup sg.

    Side 0 = first 6 pathways (slots [0,600)), side 1 = rest."""
    n = _sg_nslots(sg)
    out = []
    for side, (lo, hi) in enumerate([(0, 600), (600, n)]):
        for k in range(_sg_nchunks(sg)):
            if 256 * k < hi and 256 * (k + 1) > lo:
                out.append((k, side))
    return out


def _mm_plan():
    """Global enumeration of L1 matmuls -> w1s column base."""
    col = {}
    mi = 0
    for sg in range(NSG):
        for k, side in _sg_mms(sg):
            col[(sg, k, side)] = 256 * mi
            mi += 1
    return col, mi


MMCOL, NMM1 = _mm_plan()   # 101 matmuls


def _pack_static(inputs):
    """Pack weights/biases (shared across cores). Pure layout/folding."""
    f = lambda k: np.asarray(inputs[k], np.float32)
    w1, b1, m1 = f("w1"), f("b1"), f("m1")
    b2, b3, b4 = f("b2"), f("b3"), f("b4")
    wc, bc = f("wc"), f("bc")

    def fold(gamma, beta, rm, rv):
        s = gamma / np.sqrt(rv + 1e-5)
        return s, beta - rm * s

    s1, t1 = fold(f("gamma1"), f("beta1"), f("rm1"), f("rv1"))
    s2, t2 = fold(f("gamma2"), f("beta2"), f("rm2"), f("rv2"))
    s3, t3 = fold(f("gamma3"), f("beta3"), f("rm3"), f("rv3"))
    w1m = w1 * m1 * s1[:, None]
    b1f = b1 * s1 + t1
    w2m = f("w2") * f("m2") * s2[:, None]
    b2f = b2 * s2 + t2
    w3m = f("w3") * f("m3") * s3[:, None]
    b3f = b3 * s3 + t3
    w4m = f("w4") * f("m4")

    # pathway p -> its gene rows (from m1's structure)
    genes = [np.nonzero(m1[U1 * p] != 0)[0] for p in range(NPATH)]
    for g in genes:
        assert len(g) == KGENES

    # global gathered slot table: supergroup sg at slots [1280sg, +1280)
    slot_gene = np.zeros(NSG * 128 * MAXBLK, np.int64)
    sg_lists = []
    for sg in range(NSG):
        lg = np.concatenate([genes[p] for p in _sg_paths(sg)])
        sg_lists.append(lg)
        slot_gene[1280 * sg : 1280 * sg + len(lg)] = lg

    # L1 DoubleRow stationary fp8 [128, NMM1*256]: matmul m=(sg,k,side)
    # at cols [256m, +256) viewed as [128 part, 2 ko, 128 unit].
    w1s = np.zeros((128, NMM1 * 256), np.float32)
    b1v = np.zeros((128, NG), np.float32)
    for sg in range(NSG):
        paths = list(_sg_paths(sg))
        lg = sg_lists[sg]
        for k, side in _sg_mms(sg):
            base = MMCOL[(sg, k, side)]
            for j in range(2):
                for r in range(128):
                    s = 256 * k + 128 * j + r
                    if s >= len(lg):
                        continue
                    pi = s // KGENES
                    if pi // GP != side:
                        continue
                    p = paths[pi]
                    u0 = U1 * (pi % GP)
                    w1s[r, base + 128 * j + u0 : base + 128 * j + u0 + U1] = (
                        w1m[U1 * p : U1 * p + U1, lg[s]] * W1SCALE)
    for g in range(NG):
        for pi, p in enumerate(range(GP * g, min(GP * g + GP, NPATH))):
            b1v[U1 * pi : U1 * pi + U1, g] = b1f[U1 * p : U1 * p + U1] * W1SCALE

    # L2 DoubleRow stationary fp8 [128, NSG*256]: supergroup sg at cols
    # [256sg, +256) viewed as [128, 2, 128]: [:, side, :] maps h1 tile
    # (2sg+side) rows (20*pi+u) -> h2 tile sg rows (10*qi+v), scaled by
    # W2SCALE (h1 is stored x64, so h2 psum = 4096 * true preact).
    w2s = np.zeros((128, NSG * 256), np.float32)
    b2v = np.zeros((128, NSG), np.float32)
    for sg in range(NSG):
        for side in range(2):
            g = 2 * sg + side
            for pi, p in enumerate(range(GP * g, min(GP * g + GP, NPATH))):
                qi = p - 12 * sg
                blk = w2m[U2 * p : U2 * p + U2, U1 * p : U1 * p + U1] * W2SCALE
                w2s[U1 * pi : U1 * pi + U1,
                    256 * sg + 128 * side + U2 * qi :
                    256 * sg + 128 * side + U2 * qi + U2] = blk.T
    for sg in range(NSG):
        for qi, p in enumerate(_sg_paths(sg)):
            b2v[U2 * qi : U2 * qi + U2, sg] = b2f[U2 * p : U2 * p + U2] * H2SCALE

    # L3 stationary bf16 [128, NSG*128]: h2 rows (10*qi+v) -> h3 tile
    # pr=sg//2 rows (5*ri+w); divided by H2SCALE to undo the h2 scale.
    w3s = np.zeros((128, NSG * 128), np.float32)
    b3v = np.zeros((128, NPR), np.float32)
    for sg in range(NSG):
        pr = sg // 2
        for p in _sg_paths(sg):
            qi = p - 12 * sg
            ri = p - 24 * pr
            blk = w3m[U3 * p : U3 * p + U3, U2 * p : U2 * p + U2] / H2SCALE
            w3s[U2 * qi : U2 * qi + U2,
                128 * sg + U3 * ri : 128 * sg + U3 * ri + U3] = blk.T
    for pr in range(NPR):
        for p in range(24 * pr, min(24 * pr + 24, NPATH)):
            ri = p - 24 * pr
            b3v[U3 * ri : U3 * ri + U3, pr] = b3f[U3 * p : U3 * p + U3]

    # L4 stationary bf16 [128, NPR*128]: h3 tile pr rows (5*ri+w) ->
    # scores tile T=0 (pathways 0-119) or T=1 (120-199), row p-120T.
    w4s = np.zeros((128, NPR * 128), np.float32)
    b4v = np.zeros((128, 2), np.float32)
    for pr in range(NPR):
        T = 0 if pr < 5 else 1
        for p in range(24 * pr, min(24 * pr + 24, NPATH)):
            ri = p - 24 * pr
            w4s[U3 * ri : U3 * ri + U3, 128 * pr + p - 120 * T] = (
                w4m[p, U3 * p : U3 * p + U3])
    b4v[:120, 0] = b4[:120]
    b4v[:80, 1] = b4[120:]

    # classifier stationary bf16 [128, 2*64]
    wcs = np.zeros((128, 2 * 64), np.float32)
    wcs[:120, :NL] = wc[:, :120].T
    wcs[:80, 64 : 64 + NL] = wc[:, 120:].T
    bcv = np.zeros((128, 1), np.float32)
    bcv[:NL, 0] = bc

    bf = lambda a: np.asarray(a, np.dtype(mybir.dt.np(BF16)))
    shared = {
        "w1s": np.ascontiguousarray(w1s, dtype=NP_FP8),
        "w2s": np.ascontiguousarray(w2s, dtype=NP_FP8),
        "w3s": bf(w3s), "w4s": bf(w4s), "wcs": bf(wcs),
        "b1v": b1v, "b2v": b2v, "b3v": b3v, "b4v": b4v, "bcv": bcv,
    }
    return shared, slot_gene


def _pack(inputs):
    """Host-side packing: folded weights + per-core pre-gathered fp8 x."""
    shared, slot_gene = _pack_static(inputs)

    x8 = np.asarray(np.asarray(inputs["x"], np.float32), NP_FP8)
    xt8 = np.ascontiguousarray(x8.T)               # [G, B] fp8
    xg_all = xt8[slot_gene]                        # [NSG*1280, B]
    # -> per core [NT, NSG, 128, MAXBLK*NB]
    xg6 = xg_all.reshape(NSG, MAXBLK, 128, NCORES, NT, NB)
    in_maps = []
    for c in range(NCORES):
        m = dict(shared)
        m["xg"] = np.ascontiguousarray(
            xg6[:, :, :, c].transpose(3, 0, 2, 1, 4)).reshape(
                NT, NSG, 128, MAXBLK * NB)
        in_maps.append(m)
    return in_maps


def _assemble_out(arr):
    """Device out [NT, 64, NB] (label-major) -> [BC, NL]."""
    a = np.asarray(arr)
    return np.concatenate([a[nt, :NL, :].T for nt in range(NT)], axis=0)


def _build(repeat=None):
    """Build + compile the per-core Bass program (shared across cores).

    repeat: if set, wrap the whole compute body in an on-device For_i loop
    (used only for timing measurements; outputs are identical)."""
    nc = bacc.Bacc("TRN2", target_bir_lowering=False, debug=False,
                   enable_asserts=False)

    dram_in = {}
    for name, shape, dt_ in [
        ("xg", [NT, NSG, 128, MAXBLK * NB], FP8),
        ("w1s", [128, NMM1 * 256], FP8),
        ("w2s", [128, NSG * 256], FP8),
        ("w3s", [128, NSG * 128], BF16),
        ("w4s", [128, NPR * 128], BF16), ("wcs", [128, 2 * 64], BF16),
        ("b1v", [128, NG], F32), ("b2v", [128, NSG], F32),
        ("b3v", [128, NPR], F32), ("b4v", [128, 2], F32),
        ("bcv", [128, 1], F32),
    ]:
        dram_in[name] = nc.dram_tensor(name, shape, dt_, kind="ExternalInput").ap()
    out_d = nc.dram_tensor("out", [NT, 64, NB], F32, kind="ExternalOutput").ap()

    with tile.TileContext(nc) as tc:
        const = tc.alloc_tile_pool(name="const", bufs=1, space="SBUF")
        cs = {}
        for name, ap in dram_in.items():
            if name == "xg":
                continue
            t = const.tile(ap.shape, ap.dtype, name=f"c_{name}")
            nc.sync.dma_start(t[:], ap[:])
            cs[name] = t

        lookahead = int(os.environ.get("PIPE_SG", "1"))
        gpool = tc.alloc_tile_pool(name="gath", bufs=2 * (lookahead + 1),
                                   space="SBUF")
        h1p = tc.alloc_tile_pool(name="h1", bufs=2 * (lookahead + 1) + 1,
                                 space="SBUF")
        h2p = tc.alloc_tile_pool(name="h2", bufs=4, space="SBUF")
        h3p = tc.alloc_tile_pool(name="h3", bufs=3, space="SBUF")
        scp = tc.alloc_tile_pool(name="sc", bufs=4, space="SBUF")
        otp = tc.alloc_tile_pool(name="ot", bufs=2, space="SBUF")
        ps1 = tc.alloc_tile_pool(name="ps1", bufs=2, space="PSUM")
        ps2 = tc.alloc_tile_pool(name="ps2", bufs=2, space="PSUM")
        ps3 = tc.alloc_tile_pool(name="ps3", bufs=2, space="PSUM")
        ps4 = tc.alloc_tile_pool(name="ps4", bufs=2, space="PSUM")

        def bias_relu(dst, src, bias, on_vector):
            if on_vector:
                nc.vector.tensor_scalar(dst, src, bias, 0.0,
                                        mybir.AluOpType.add,
                                        mybir.AluOpType.max)
            else:
                nc.scalar.activation(dst, src, RELU, bias=bias)

        loop_kw = {}
        if os.environ.get("LOOP_STAGGER"):
            loop_kw["staggered_reset"] = True
        if os.environ.get("LOOP_HINT"):
            from concourse.engine_type import EngineType
            loop_kw["hint_engines"] = (EngineType.PE, EngineType.DVE,
                                       EngineType.Activation, EngineType.SP)
        unroll = int(os.environ.get("LOOP_UNROLL", "1")) if repeat else 1
        if repeat:
            assert repeat % unroll == 0
        loop = (tc.For_i(0, repeat // unroll, 1, **loop_kw) if repeat
                else contextlib.nullcontext())
        with loop:
          for _unroll_i in range(unroll):
            sc_tiles = {}      # (nt, T) -> tile
            p3t = {}
            p4t = {}
            h1pairs = {}       # sg -> [h1pair_nt0, h1pair_nt1]

            def emit_l1(sg):
                """DMA + L1 DoubleRow matmuls + h1 activations for sg."""
                nblk = 2 * _sg_nchunks(sg)
                mms = _sg_mms(sg)
                gts = []
                h1pair = []
                for nt in range(NT):
                    gt = gpool.tile([128, MAXBLK * NB], FP8, name="gt", tag="gt")
                    nc.sync.dma_start(gt[:, : nblk * NB],
                                      dram_in["xg"][nt, sg, :, : nblk * NB])
                    gts.append(gt)
                    h1pair.append(h1p.tile([128, 2 * NB], FP8,
                                           name="h1t", tag="h1t"))
                h1pairs[sg] = h1pair
                for side in range(2):
                    g = 2 * sg + side
                    ks = [k for k, s in mms if s == side]
                    for nt in range(NT):
                        p1 = ps1.tile([128, NB], F32, name="p1", tag="p1")
                        for i, k in enumerate(ks):
                            cb = MMCOL[(sg, k, side)]
                            lhsT = cs["w1s"][:, cb : cb + 256].rearrange(
                                "p (two u) -> p two u", two=2)
                            rhs = gts[nt][:, 2 * k * NB : (2 * k + 2) * NB
                                          ].rearrange("p (two n) -> p two n",
                                                      two=2)
                            nc.tensor.matmul(
                                p1[:], lhsT, rhs, perf_mode=DROW,
                                start=(i == 0), stop=(i == len(ks) - 1),
                            )
                        bias_relu(h1pair[nt][:, side * NB : (side + 1) * NB],
                                  p1[:], cs["b1v"][:, g : g + 1],
                                  (side + nt) % 2)

            def emit_rest(sg):
                """L2/L3/L4/scores for sg (consumes h1pairs[sg])."""
                h1pair = h1pairs.pop(sg)
                for nt in range(NT):
                    p2 = ps2.tile([128, NB], F32, name="p2", tag="p2")
                    lhsT = cs["w2s"][:, 256 * sg : 256 * (sg + 1)].rearrange(
                        "p (two u) -> p two u", two=2)
                    rhs = h1pair[nt].rearrange("p (two n) -> p two n", two=2)
                    nc.tensor.matmul(p2[:], lhsT, rhs, perf_mode=DROW,
                                     start=True, stop=True)
                    h2 = h2p.tile([128, NB], BF16, name="h2t", tag="h2t")
                    bias_relu(h2[:], p2[:], cs["b2v"][:, sg : sg + 1],
                              (sg + nt) % 2)
                    # ---- L3: accumulate pairs of supergroups ----
                    if sg % 2 == 0:
                        p3t[nt] = ps3.tile([128, NB], F32, name="p3", tag="p3")
                    last3 = (sg % 2 == 1) or (sg == NSG - 1)
                    nc.tensor.matmul(
                        p3t[nt], cs["w3s"][:, 128 * sg : 128 * (sg + 1)],
                        h2[:], start=(sg % 2 == 0), stop=last3,
                    )
                    if last3:
                        pr = sg // 2
                        h3 = h3p.tile([128, NB], BF16, name="h3t", tag="h3t")
                        bias_relu(h3[:], p3t[nt], cs["b3v"][:, pr : pr + 1],
                                  (pr + nt) % 2)
                        # ---- L4: scores tile A (pr 0-4) / B (pr 5-8) ----
                        T = 0 if pr < 5 else 1
                        first4 = pr in (0, 5)
                        if first4:
                            p4t[nt] = ps4.tile([128, NB], F32, name="p4",
                                               tag="p4")
                        nc.tensor.matmul(
                            p4t[nt], cs["w4s"][:, 128 * pr : 128 * (pr + 1)],
                            h3[:], start=first4, stop=(pr in (4, NPR - 1)),
                        )
                        if pr in (4, NPR - 1):
                            sc = scp.tile([128, NB], BF16, name="sct",
                                          tag="sct")
                            bias_relu(sc[:], p4t[nt],
                                      cs["b4v"][:, T : T + 1], (T + nt) % 2)
                            sc_tiles[(nt, T)] = sc

            # software-pipelined emission: L2+ of sg trails L1 of sg+LOOKAHEAD
            # so PE never parks long on an activation semaphore.
            for sg in range(NSG):
                emit_l1(sg)
                if sg >= lookahead:
                    emit_rest(sg - lookahead)
            for sg in range(NSG - lookahead, NSG):
                emit_rest(sg)
            # ---- classifier (label-major out; host transposes) ----
            for nt in range(NT):
                pc = ps1.tile([128, NB], F32, name="pc", tag="p1")
                for k in range(2):
                    nc.tensor.matmul(
                        pc[:64, :], cs["wcs"][:, 64 * k : 64 * (k + 1)],
                        sc_tiles[(nt, k)][:], start=(k == 0), stop=(k == 1),
                    )
                ot = otp.tile([64, NB], F32, name="ott", tag="ott")
                nc.scalar.activation(ot[:], pc[:64, :], IDENT,
                                     bias=cs["bcv"][:64, 0:1])
                nc.sync.dma_start(out_d[nt], ot[:])

        for pl in (ps4, ps3, ps2, ps1, otp, scp, h3p, h2p, h1p, gpool, const):
            pl.release()

    nc.compile()
    return nc


def get_compiled():
    global _COMPILED
    if _COMPILED is None:
        _COMPILED = _build()
    return _COMPILED


def kernel(**inputs):
    nc = get_compiled()
    in_maps = _pack(inputs)
    res = run_bass_kernel_spmd(nc, in_maps, core_ids=list(range(NCORES)))
    return np.concatenate([_assemble_out(res.results[c]["out"])
                           for c in range(NCORES)], axis=0)


if __name__ == "__main__":
    print("built", get_compiled())
